# revision 49
# baseline (speedup 1.0000x reference)
"""DGCNN forward on 8 Trainium2 NeuronCores (Bass/Tile), pure data parallel.

V3: V2 restructured to cut DVE (vector engine) time, which the V2 trace
showed as the bottleneck (78% busy; MAX8+FIND_INDEX8+MATCH_REPLACE8 = 310us
of a 535us span):

- Packed-key top-k: the ACT engine quantizes distances to fp16
  (s = relu(-dist*sc + xx_i*sc + 2^-9), per-row bias via a PE transpose of
  the -0.5*xx row), one DVE STT builds key = (CONST - bits16(s))*1024 + j
  from the fp16 BIT PATTERN (monotone in -dist, log-spaced so resolution
  concentrates at near-zero distances where the top-20 boundary lives).
  3x max8 + 2x match_replace on keys then give values AND indices
  (idx = key mod 1024) -- the three FIND_INDEX8 passes and the u16->f32
  index CAST are gone. Keys are unique (j embedded) so match_replace is
  tie-safe.
- L4 neighbor-max tree: 6 wide contiguous fp16 TT ops (q-pair folding then
  a strided f-major j-tree) instead of 30 fragmented [128,256]/[128,128]
  ops.
- LeakyReLU moved to the ACT engine (Act.Lrelu); v tiles and the z=mk+v
  add are fp16 so the add runs in DVE 2x mode.
"""

import os

import numpy as np

N = 1024
K = 20
NCORES = 8
EPS = 1e-5
SLOPE = 0.01
NEG = -3.0e38

EDGE_LAYERS = [(3, 64), (64, 64), (64, 128), (128, 256)]
# Per-layer key scales: sc = 2/D, D = observed max |dist| with headroom.
SCALES = [2.0 / 64.4, 2.0 / 410.2, 2.0 / 161.4, 2.0 / 167.6]
KCONST = 15360.0  # key = (KCONST - bits16(s))*1024 + j; top keys < 2^24
SBIAS0 = 2.0 ** -9

_CACHE = {}
LAST_RESULTS = None
DIST_F32R = bool(int(os.environ.get("DGCNN_DIST_F32R", "1")))


def _build():
    import concourse.bass as bass
    import concourse.mybir as mybir
    import concourse.tile as tile
    from concourse import bacc

    dt = mybir.dt
    f32 = dt.float32
    u16 = dt.uint16
    f16 = dt.float16
    i16 = dt.int16
    i32 = dt.int32
    f32r = dt.float32r
    Alu = mybir.AluOpType
    Act = mybir.ActivationFunctionType
    AX = mybir.AxisListType

    nc = bacc.Bacc("TRN2", target_bir_lowering=False, debug=False,
                   num_swdge_queues=4)
    b32 = lambda ap: ap.bitcast(f32)

    # ---------------- DRAM I/O ----------------
    x0in = nc.dram_tensor("x0in", [33, N], f32, kind="ExternalInput")
    xa1in = nc.dram_tensor("xa1in", [33, N], f32, kind="ExternalInput")
    ATs, BTs, c0s = [], [], []
    for li, (C, O) in enumerate(EDGE_LAYERS):
        ATs.append(nc.dram_tensor(f"AT{li}", [C, O], f32, kind="ExternalInput"))
        BTs.append(nc.dram_tensor(f"BT{li}", [C, O], f32, kind="ExternalInput"))
        c0s.append(nc.dram_tensor(f"c0{li}", [1, O], f32, kind="ExternalInput"))
    w5T = nc.dram_tensor("w5T", [512, 1024], f32, kind="ExternalInput")
    l1T = nc.dram_tensor("l1T", [1024, 512], f32, kind="ExternalInput")
    b6 = nc.dram_tensor("b6", [1, 512], f32, kind="ExternalInput")
    l2T = nc.dram_tensor("l2T", [512, 256], f32, kind="ExternalInput")
    c7 = nc.dram_tensor("c7", [1, 256], f32, kind="ExternalInput")
    l3T = nc.dram_tensor("l3T", [256, 40], f32, kind="ExternalInput")
    b8 = nc.dram_tensor("b8", [1, 40], f32, kind="ExternalInput")
    identd = nc.dram_tensor("identd", [128, 128], f32, kind="ExternalInput")
    rep16d = nc.dram_tensor("rep16d", [16, 128], f32, kind="ExternalInput")
    initrd = nc.dram_tensor("initrd", [30, 1024], f32, kind="ExternalInput")
    iotad = nc.dram_tensor("iotad", [128, 1024], f32, kind="ExternalInput")
    out_d = nc.dram_tensor("out", [1, 40], f32, kind="ExternalOutput")

    with tile.TileContext(nc) as tc, __import__("contextlib").ExitStack() as ctx:
        const = ctx.enter_context(tc.tile_pool(name="const", bufs=1))
        xpool = ctx.enter_context(tc.tile_pool(name="xpool", bufs=1))
        work = ctx.enter_context(tc.tile_pool(name="work", bufs=2))
        gth_p = ctx.enter_context(tc.tile_pool(name="gth", bufs=10))
        vt_p = ctx.enter_context(tc.tile_pool(name="vt", bufs=3))
        small = ctx.enter_context(tc.tile_pool(name="small", bufs=4))
        pdp = ctx.enter_context(tc.tile_pool(name="pdp", bufs=2, space="PSUM"))
        mm = ctx.enter_context(tc.tile_pool(name="mm", bufs=2, space="PSUM"))
        fold_p = ctx.enter_context(tc.tile_pool(name="fold", bufs=1, space="PSUM"))
        dram = ctx.enter_context(tc.tile_pool(name="dram", bufs=2, space="DRAM"))

        # ------------- constants into SBUF (f32r via bitcast DMA) -------------
        def load_r2(name, dram_t, rows, cols):
            t = const.tile([rows, cols], f32r, tag=name)
            nc.scalar.dma_start(t[:], dram_t.ap().bitcast(f32r))
            return t

        AT_sb = [load_r2(f"AT{i}", ATs[i], *ATs[i].shape) for i in range(4)]
        BT_sb = [load_r2(f"BT{i}", BTs[i], *BTs[i].shape) for i in range(4)]
        c0_sb = [load_r2(f"c0{i}", c0s[i], *c0s[i].shape) for i in range(4)]
        b6_sb = load_r2("b6", b6, 1, 512)
        c7_sb = load_r2("c7", c7, 1, 256)
        b8_sb = load_r2("b8", b8, 1, 40)
        ident_s = const.tile([128, 128], f32, tag="ident_s")
        nc.sync.dma_start(ident_s[:], identd.ap())
        rep16_s = const.tile([16, 128], f32, tag="rep16_s")
        nc.sync.dma_start(rep16_s[:], rep16d.ap())
        iota_sb = const.tile([128, 1024], f32, tag="iota_sb")
        nc.sync.dma_start(iota_sb[:], iotad.ap())
        ones1024 = const.tile([1, 1024], f32r, tag="ones1024")
        nc.sync.dma_start(ones1024[:], initrd.ap().bitcast(f32r)[29:30, :])

        # point-major u tables in SBUF (SBUF-source SWDGE gather with
        # tokens_per_rank=128: idx j reads partition j%128, slot j//128)
        u0t = xpool.tile([128, 1024], f16, tag="u0")
        u1t = xpool.tile([128, 1024], f16, tag="u1")
        u2t = xpool.tile([128, 1024], f16, tag="u2")
        u3t = xpool.tile([128, 2048], f16, tag="u3t")
        u_sb = [u0t, u1t, u2t, u3t]

        def make_emit_u(li):
            C, O = EDGE_LAYERS[li]
            Opad = max(O, 128)

            def emit_u_pair(xch_next, qsl):
                uc = (lambda ap: ap) if O >= 256 else b32
                for mu in (qsl.start // 128, qsl.start // 128 + 1):
                    usl = slice(mu * 128, (mu + 1) * 128)
                    put = mm.tile([128, 512], f32, tag="mm")
                    pu = put[:, 0:O]
                    nc.tensor.matmul(pu, uc(xch_next[0:C, usl]),
                                     uc(AT_sb[li][:]), start=True, stop=False)
                    nc.tensor.matmul(pu, uc(ones_row[:]), uc(c0_sb[li][:]),
                                     start=False, stop=True)
                    nc.scalar.activation(
                        u_sb[li][:, mu * Opad:mu * Opad + O], pu, Act.Copy)

            return emit_u_pair

        # persistent channel-major feature tiles (f32r). lhsT tiles carry an
        # all-ones row after the feature rows (pairs with the rhs nxx row).
        x0 = xpool.tile([33, 1024], f32r, tag="x0")
        xar1 = xpool.tile([33, 1024], f32r, tag="xar1")
        x1 = xpool.tile([65, 1024], f32r, tag="x1")
        x2 = xpool.tile([65, 1024], f32r, tag="x2")
        x3 = xpool.tile([128, 1024], f32r, tag="x3")
        x4a = xpool.tile([128, 1024], f32r, tag="x4a")
        x4b = xpool.tile([128, 1024], f32r, tag="x4b")
        gp = xpool.tile([128, 8], f32, tag="gp")

        initr = initrd.ap().bitcast(f32r)
        nc.sync.dma_start(x0[:], x0in.ap().bitcast(f32r))
        nc.sync.dma_start(xar1[:], xa1in.ap().bitcast(f32r))
        # ones for rank-1 matmul tricks, f32r (initrd row 29 is all-ones)
        ones_row = const.tile([1, 128], f32r, tag="ones_row")
        nc.sync.dma_start(ones_row[:], initrd.ap().bitcast(f32r)[29:30, 0:128])
        ones_col = const.tile([128, 1], f32r, tag="ones_col")
        nc.sync.dma_start(ones_col[:],
                          initrd.ap().bitcast(f32r)[29:30, 0:128].rearrange("o c -> c o"))
        nc.sync.dma_start(x1[64:65, :], initr[29:30, :])  # ones row
        nc.sync.dma_start(x2[64:65, :], initr[29:30, :])  # ones row

        # ---------------- per-chunk prep for the NEXT layer ----------------
        def make_prep(xch_next, C, rhs_tile, emit_u_next=None):
            """Returns (per-chunk prep fn, nxxt-or-None). Emitted inside the
            previous layer's phase E so xsq/xar/colsum/nxx/u start as soon as
            each x chunk is written (engine streams are in-order)."""
            aug = C < 128
            xsq = work.tile([C, 1024], f32r, tag=f"xsq{C}", bufs=1)
            nxxt = None if aug else work.tile([1, 1024], f32r, tag="nxx", bufs=1)
            nxrow = (C if C >= 32 else 32) if aug else None

            def prep(mc):
                msl = slice(mc * 128, (mc + 1) * 128)
                nc.scalar.activation(xsq[0:C, msl], xch_next[0:C, msl],
                                     Act.Square)
                if aug:
                    nc.scalar.activation(rhs_tile[0:C, msl],
                                         xch_next[0:C, msl], Act.Copy)
                if mc % 2 == 1:
                    qsl = slice((mc - 1) * 128, (mc + 1) * 128)
                    pst = mm.tile([128, 512], f32, tag="mm")
                    ps = pst[0:1, 0:256]
                    nc.tensor.matmul(ps, b32(ones_col[0:C, :]),
                                     b32(xsq[0:C, qsl]))
                    dst = (rhs_tile[nxrow:nxrow + 1, qsl]
                           if aug else nxxt[0:1, qsl])
                    nc.scalar.activation(dst, ps, Act.Copy, scale=-0.5)
                    if emit_u_next is not None:
                        emit_u_next(xch_next, qsl)

            return prep, nxxt

        # ---------------- edge conv layer ----------------
        def edge_layer(li, xch, C, O, xouts, rhs_tile, nxxt=None, prep=None,
                       emit_u_self=None):
            """xch: lhsT tile ([C(+ones) rows, 1024], f32r).
            rhs_tile: None for L4 (uses xch + rank-1), else the rhs tile,
            filled by the previous layer's prep (L1: host-prebuilt xar1).
            prep: per-chunk prep fn for the NEXT layer, called in phase E.
            xouts: [(tile, rows)] per 128-channel output block."""
            Opad = max(O, 128)
            Of = Opad // 128
            aug = C < 128
            sc = SCALES[li]
            if aug:
                nxrow = 32 if li == 0 else 64
                nxap = rhs_tile[nxrow:nxrow + 1, :]
            else:
                nxrow = 0
                nxap = nxxt[0:1, :]
            id1 = ident_s[nxrow:nxrow + 1, nxrow:nxrow + 1]

            # v channel-major [O, 1024] (f16); deferred emission (phase E use)
            vs = []

            def emit_v():
                for f in range(Of if O >= 128 else 1):
                    osl = slice(f * 128, min((f + 1) * 128, O))
                    orows = osl.stop - osl.start
                    vt = vt_p.tile([128, 1024], f16, tag="vt")
                    for h in range(2):
                        nsl = slice(h * 512, (h + 1) * 512)
                        pv = mm.tile([128, 512], f32, tag="mm")
                        nc.tensor.matmul(pv[0:orows, :], BT_sb[li][:, osl],
                                         xch[0:C, nsl])
                        nc.scalar.activation(vt[0:orows, nsl], pv[0:orows, :],
                                             Act.Copy)
                    vs.append(vt)

            gq_tiles = {}

            def emit_gather(m):
                gq = gth_p.tile([128, Of * 2560], f16, tag=f"gth{Of}", bufs=4 if Of == 1 else 2)
                it, ioff = idx_tiles[m]
                for q in range(4):
                    dst = gq[:, q * Of * 640:(q + 1) * Of * 640]
                    nc.gpsimd.dma_gather(
                        dst.rearrange("p (f i) -> p f i", f=Of),
                        u_sb[li][:],
                        it[:, ioff + q * 40:ioff + (q + 1) * 40],
                        640, 640, Opad, transpose=True,
                        queue_num=q,
                        sbuf_tokens_per_rank=128,
                        sbuf_free_dim_per_rank=Opad * 2,
                    )
                gq_tiles[m] = gq

            # ---- phase E body: neighbor max (wide fp16 tree on DVE),
            #      z = mk + v (DVE, fp16 2x), lrelu on ACT; called one PAIR
            #      behind the topk loop so gq tiles recycle without a stall ----
            def phase_e(m):
                csl = slice(m * 128, (m + 1) * 128)
                gq = gq_tiles[m]

                def vmax(out, a, b):
                    nc.vector.tensor_tensor(out=out, in0=a, in1=b, op=Alu.max)

                if Of == 1:
                    # [128, 2560] = 20 contiguous j-slices of 128; wide tree
                    mkT = small.tile([128, 128], f16, tag="mkT")
                    a1 = small.tile([128, 1024], f16, tag="a1", bufs=2)
                    vmax(a1[:], gq[:, 0:1024], gq[:, 1024:2048])
                    vmax(a1[0:128, 0:512], a1[:, 0:512], a1[:, 512:1024])
                    vmax(a1[0:128, 0:256], a1[:, 0:256], a1[:, 256:512])
                    b1 = small.tile([128, 256], f16, tag="b1")
                    vmax(b1[:], gq[:, 2048:2304], gq[:, 2304:2560])
                    vmax(b1[0:128, 0:128], b1[:, 0:128], b1[:, 128:256])
                    vmax(a1[0:128, 0:128], a1[:, 0:128], a1[:, 128:256])
                    vmax(mkT[:], a1[:, 0:128], b1[:, 0:128])
                else:
                    # q-blocks [f2, j5, i128] of 1280: fold q-pairs with three
                    # wide ops, then a strided f-major j-tree (5 -> 1).
                    mkT = small.tile([128, 256], f16, tag="mk2")
                    a4 = small.tile([128, 1280], f16, tag="a4", bufs=1)
                    b4 = small.tile([128, 1280], f16, tag="b4", bufs=1)
                    vmax(a4[:], gq[:, 0:1280], gq[:, 1280:2560])
                    vmax(b4[:], gq[:, 2560:3840], gq[:, 3840:5120])
                    vmax(a4[:], a4[:], b4[:])
                    av = a4[:].rearrange("p (f j) -> p f j", f=2)
                    r1 = b4[:, 0:512].rearrange("p (f x) -> p f x", f=2)
                    vmax(r1, av[:, :, 0:256], av[:, :, 256:512])
                    r2 = b4[:, 512:768].rearrange("p (f x) -> p f x", f=2)
                    vmax(r2, r1[:, :, 0:128], r1[:, :, 128:256])
                    mkv = mkT[:].rearrange("p (f x) -> p f x", f=2)
                    vmax(mkv, r2, av[:, :, 512:640])

                for f, (xt, rows) in enumerate(xouts):
                    z = small.tile([128, 128], f16, tag="z")
                    nc.vector.tensor_add(z[0:rows, :],
                                         mkT[0:rows, f * 128:f * 128 + 128],
                                         vs[f][0:rows, csl])
                    nc.scalar.activation(xt[0:rows, csl], z[0:rows, :],
                                         Act.Lrelu, alpha=SLOPE)
                if prep is not None:
                    prep(m)

            # ---- phase B: dist + packed-key topk, two chunks interleaved ----
            idx_tiles = []
            dc = (lambda ap: ap) if DIST_F32R else (lambda ap: ap.bitcast(f32))

            def emit_dist_mm(m, btcol):
                csl = slice(m * 128, (m + 1) * 128)
                # per-row bias: sbias_i = xx_i*sc + 2^-9 from the -0.5*xx row
                nc.tensor.transpose(btcol, b32(nxap[0:1, csl]), id1)
                sbias = small.tile([128, 1], f32, tag="sb", bufs=4)
                nc.scalar.activation(sbias[:], btcol, Act.Copy,
                                     scale=-2.0 * sc, bias=SBIAS0)
                pd = pdp.tile([128, 1024], f32, tag="pd")
                s16 = work.tile([128, 1024], f16, tag="s16", bufs=2)
                for h in range(2):
                    nsl = slice(h * 512, (h + 1) * 512)
                    if aug:
                        nc.tensor.matmul(pd[:, nsl], dc(xch[:, csl]),
                                         dc(rhs_tile[:, nsl]))
                    else:
                        nc.tensor.matmul(pd[:, nsl], dc(xch[0:C, csl]),
                                         dc(xch[0:C, nsl]), start=True, stop=False)
                        nc.tensor.matmul(pd[:, nsl], dc(ones_row[:]),
                                         dc(nxxt[0:1, nsl]), start=False, stop=True)
                    nc.scalar.activation(s16[:, nsl], pd[:, nsl], Act.Relu,
                                         scale=-2.0 * sc, bias=sbias[:, 0:1])
                return s16

            def emit_keys(s16):
                keys = work.tile([128, 1024], f32, tag="keys", bufs=3)
                nc.vector.scalar_tensor_tensor(
                    out=keys[:], in0=s16[:].bitcast(u16), scalar=-1024.0,
                    in1=iota_sb[:], op0=Alu.mult, op1=Alu.add)
                return keys

            def emit_idx(m, idxf):
                # fold, permute-evac, replicate, convert (wrap order for SWDGE)
                psF = fold_p.tile([16, 160], f32, tag="psF")
                for t in range(8):
                    nc.tensor.matmul(psF[:, t * 20:(t + 1) * 20],
                                     ident_s[:, t * 16:(t + 1) * 16], idxf)
                wf = small.tile([16, 160], f32, tag="wf", bufs=2)
                nc.scalar.activation(
                    wf[:].rearrange("p (j h) -> p h j", h=8),
                    psF[:].rearrange("p (h j) -> p h j", h=8), Act.Copy)
                psRt = mm.tile([128, 512], f32, tag="mm")
                psR = psRt[:, 0:160]
                nc.tensor.matmul(psR, rep16_s[:], wf[:])
                idxw = work.tile([128, 160], i16, tag="idxw", bufs=9)
                nc.scalar.activation(idxw[:], psR, Act.Copy)
                idx_tiles.append((idxw, 0))
                emit_gather(m)

            pend = None
            for mp in range(4):
                m0, m1 = 2 * mp, 2 * mp + 1
                if mp == 0:
                    bt = fold_p.tile([128, 4], f32, tag="ph")
                    pend = (emit_dist_mm(m0, bt[:, 0:1]),
                            emit_dist_mm(m1, bt[:, 1:2]))
                keys0 = emit_keys(pend[0])
                keys1 = emit_keys(pend[1])
                m48 = small.tile([128, 48], f32, tag="m48", bufs=4)
                for r in range(3):
                    rsl0 = slice(r * 8, (r + 1) * 8)
                    rsl1 = slice(24 + r * 8, 24 + (r + 1) * 8)
                    nc.vector.max(m48[:, rsl0], keys0[:])
                    nc.vector.max(m48[:, rsl1], keys1[:])
                    if r < 2:
                        nc.vector.match_replace(keys0[:], m48[:, rsl0],
                                                keys0[:], NEG)
                        nc.vector.match_replace(keys1[:], m48[:, rsl1],
                                                keys1[:], NEG)
                # hoist the next pair's dist matmuls + fp16 quantize (PE/ACT
                # only) ahead of this pair's idx/tree ACT work so s16 is
                # ready when the next keys-STT issues on the DVE
                if mp < 3:
                    btn = fold_p.tile([128, 4], f32, tag="ph")
                    pend = (emit_dist_mm(m0 + 2, btn[:, 0:1]),
                            emit_dist_mm(m1 + 2, btn[:, 1:2]))
                # idx = key & 1023 via exact i32 cast (keys are exact ints);
                # both chunks' 20 indices extracted in one 3-op sequence
                idxf2 = small.tile([128, 40], f32, tag="idxf", bufs=4)
                ti = small.tile([128, 40], i32, tag="ti", bufs=4)
                nc.vector.tensor_copy(
                    ti[:].rearrange("p (c j) -> p c j", c=2),
                    m48[:].rearrange("p (c x) -> p c x", c=2)[:, :, 0:20])
                nc.vector.tensor_scalar(ti[:], ti[:], 1023, None,
                                        op0=Alu.bitwise_and)
                nc.vector.tensor_copy(idxf2[:], ti[:])
                idxf0 = idxf2[:, 0:20]
                idxf1 = idxf2[:, 20:40]
                if mp == 0 and emit_u_self is not None:
                    for up in range(4):
                        emit_u_self(xch, slice(up * 256, (up + 1) * 256))
                emit_idx(m0, idxf0)
                emit_idx(m1, idxf1)
                if mp == 0:
                    emit_v()
                if mp >= 1:
                    phase_e(2 * (mp - 1))
                    phase_e(2 * mp - 1)
            phase_e(6)
            phase_e(7)

        xar2 = xpool.tile([65, 1024], f32r, tag="xar2")
        xar3 = xpool.tile([65, 1024], f32r, tag="xar3")
        prep2, _ = make_prep(x1, 64, xar2, emit_u_next=make_emit_u(1))
        edge_layer(0, x0, 3, 64, [(x1, 64)], xar1, prep=prep2,
                   emit_u_self=make_emit_u(0))
        prep3, _ = make_prep(x2, 64, xar3, emit_u_next=make_emit_u(2))
        edge_layer(1, x1, 64, 64, [(x2, 64)], xar2, prep=prep3)
        prep4, nxxt4 = make_prep(x3, 128, None, emit_u_next=make_emit_u(3))
        edge_layer(2, x2, 64, 128, [(x3, 128)], xar3, prep=prep4)

        # conv5 weights + head weights (loaded during L3/L4)
        w5_rows = [(0, 64), (64, 128), (128, 256), (256, 384), (384, 512)]
        w5_sb = []
        for i, (r0, r1) in enumerate(w5_rows):
            t = const.tile([r1 - r0, 1024], f32r, tag=f"w5_{i}")
            nc.sync.dma_start(t[:], w5T.ap().bitcast(f32r)[r0:r1, :])
            w5_sb.append(t)

        # conv5 emitted inside L4's phase E: half 0 after chunk 3's features,
        # half 1 after chunk 7 — PE chews it while DVE finishes L4 topk/trees
        xc_full = [(x1, 64), (x2, 64), (x3, 128), (x4a, 128), (x4b, 128)]
        gph = xpool.tile([128, 32], f32, tag="gph")

        def conv5_q(q):
            nsl = slice(q * 256, (q + 1) * 256)
            for mo in range(8):
                msl = slice(mo * 128, (mo + 1) * 128)
                pet = mm.tile([128, 512], f32, tag="mm")
                pe = pet[:, 0:256]
                for k in range(5):
                    nc.tensor.matmul(pe, w5_sb[k][:, msl],
                                     xc_full[k][0][0:xc_full[k][1], nsl],
                                     start=(k == 0), stop=(k == 4))
                nc.vector.reduce_max(gph[:, q * 8 + mo:q * 8 + mo + 1], pe,
                                     axis=AX.X)

        def prep5(mc):
            if mc % 2 == 1:
                conv5_q(mc // 2)

        edge_layer(3, x3, 128, 256, [(x4a, 128), (x4b, 128)], None,
                   nxxt=nxxt4, prep=prep5)
        gpt = small.tile([128, 16], f32, tag="gpt", bufs=1)
        nc.vector.tensor_tensor(out=gpt[:, 0:8], in0=gph[:, 0:8],
                                in1=gph[:, 8:16], op=Alu.max)
        nc.vector.tensor_tensor(out=gpt[:, 8:16], in0=gph[:, 16:24],
                                in1=gph[:, 24:32], op=Alu.max)
        nc.vector.tensor_tensor(out=gp[:, 0:8], in0=gpt[:, 0:8],
                                in1=gpt[:, 8:16], op=Alu.max)

        l1_sb = []
        for k in range(8):
            t = const.tile([128, 512], f32r, tag=f"l1_{k}")
            nc.sync.dma_start(t[:], l1T.ap().bitcast(f32r)[k * 128:(k + 1) * 128, :])
            l1_sb.append(t)
        l2_sb = []
        for k in range(4):
            t = const.tile([128, 256], f32r, tag=f"l2_{k}")
            nc.sync.dma_start(t[:], l2T.ap().bitcast(f32r)[k * 128:(k + 1) * 128, :])
            l2_sb.append(t)
        l3_sb = []
        for k in range(2):
            t = const.tile([128, 40], f32r, tag=f"l3_{k}")
            nc.sync.dma_start(t[:], l3T.ap().bitcast(f32r)[k * 128:(k + 1) * 128, :])
            l3_sb.append(t)

        # ---------------- MLP head: row-form GEMV (f32r, N=512/256) ----------
        gpr = small.tile([128, 8], f32r, tag="gpr", bufs=1)
        nc.scalar.activation(gpr[:], gp[:], Act.Copy)

        p1t = mm.tile([128, 512], f32, tag="mm")
        p1r = p1t[0:1, :]
        for k in range(8):
            nc.tensor.matmul(p1r, gpr[:, k:k + 1], l1_sb[k][:],
                             start=(k == 0), stop=False)
        nc.tensor.matmul(p1r, ones_row[0:1, 0:1], b6_sb[:],
                         start=False, stop=True)
        y1l = small.tile([1, 512], f32, tag="y1l", bufs=1)
        nc.scalar.activation(y1l[:], p1r, Act.Lrelu, alpha=SLOPE)
        p1c = fold_p.tile([128, 4], f32, tag="ph")
        for j in range(4):
            nc.tensor.transpose(p1c[:, j:j + 1], y1l[0:1, j * 128:(j + 1) * 128],
                                ident_s[0:1, 0:1])
        y1c = small.tile([128, 4], f32r, tag="y1c", bufs=1)
        nc.scalar.activation(y1c[:], p1c[:], Act.Copy)

        p2t = mm.tile([128, 512], f32, tag="mm")
        p2r = p2t[0:1, 0:256]
        for k in range(4):
            nc.tensor.matmul(p2r, y1c[:, k:k + 1], l2_sb[k][:],
                             start=(k == 0), stop=False)
        nc.tensor.matmul(p2r, ones_row[0:1, 0:1], c7_sb[:],
                         start=False, stop=True)
        y2l = small.tile([1, 256], f32, tag="y2l", bufs=1)
        nc.scalar.activation(y2l[:], p2r, Act.Lrelu, alpha=SLOPE)
        p2c = fold_p.tile([128, 4], f32, tag="ph")
        for j in range(2):
            nc.tensor.transpose(p2c[:, j:j + 1], y2l[0:1, j * 128:(j + 1) * 128],
                                ident_s[0:1, 0:1])
        y2c = small.tile([128, 2], f32, tag="y2c", bufs=1)
        nc.scalar.activation(y2c[:], p2c[:, 0:2], Act.Copy)

        p3t = mm.tile([128, 512], f32, tag="mm")
        p3r = p3t[0:1, 0:40]
        for k in range(2):
            nc.tensor.matmul(p3r, y2c[:, k:k + 1], b32(l3_sb[k][:, 0:40]),
                             start=(k == 0), stop=False)
        nc.tensor.matmul(p3r, b32(ones_row[0:1, 0:1]), b32(b8_sb[:]),
                         start=False, stop=True)
        y3 = small.tile([1, 40], f32, tag="y3", bufs=1)
        nc.scalar.activation(y3[:], p3r, Act.Copy)
        nc.sync.dma_start(out_d.ap(), y3[:])

    nc.compile()
    return nc


def _prep_inputs(inputs):
    """Fold eval-mode BN into conv/linear weights; transpose for the device."""
    f = np.float32
    s = lambda g: (g / np.sqrt(f(1.0) + f(EPS))).astype(f)

    def edge(w, g, b, bias=None):
        O, C2 = w.shape
        C = C2 // 2
        sc = s(g)
        Wd = w[:, :C]
        Wc = w[:, C:]
        A = sc[:, None] * Wd
        Bm = sc[:, None] * (Wc - Wd)
        c0 = sc * (bias if bias is not None else 0.0) + b
        return A.T.copy().astype(f), Bm.T.copy().astype(f), c0.reshape(1, -1).astype(f)

    d = {}
    d["AT0"], d["BT0"], d["c00"] = edge(inputs["conv1_w"], inputs["bn1_g"],
                                        inputs["bn1_b"], inputs["conv1_b"])
    d["AT1"], d["BT1"], d["c01"] = edge(inputs["conv2_w"], inputs["bn2_g"], inputs["bn2_b"])
    d["AT2"], d["BT2"], d["c02"] = edge(inputs["conv3_w"], inputs["bn3_g"], inputs["bn3_b"])
    d["AT3"], d["BT3"], d["c03"] = edge(inputs["conv4_w"], inputs["bn4_g"], inputs["bn4_b"])
    d["w5T"] = inputs["conv5_w"].T.copy().astype(f)
    s6 = s(inputs["bn6_g"])
    d["l1T"] = (s6[:, None] * inputs["lin1_w"]).T.copy().astype(f)
    d["b6"] = inputs["bn6_b"].reshape(1, -1).astype(f)
    s7 = s(inputs["bn7_g"])
    d["l2T"] = (s7[:, None] * inputs["lin2_w"]).T.copy().astype(f)
    d["c7"] = (s7 * inputs["lin2_b"] + inputs["bn7_b"]).reshape(1, -1).astype(f)
    d["l3T"] = inputs["lin3_w"].T.copy().astype(f)
    d["b8"] = inputs["lin3_b"].reshape(1, -1).astype(f)
    d["identd"] = np.eye(128, dtype=f)
    rep = np.zeros((16, 128), f)
    for m in range(128):
        rep[m % 16, m] = 1.0
    d["rep16d"] = rep
    initr = np.zeros((30, 1024), f)
    initr[29, :] = 1.0
    d["initrd"] = initr
    iota = (KCONST * 1024.0 + np.arange(1024)).astype(f)
    d["iotad"] = np.broadcast_to(iota, (128, 1024)).copy()
    return d


def _install_ntff_hook():
    import sys
    import types

    if "antenv.axon_hooks" in sys.modules:
        return
    import antenv

    mod = types.ModuleType("antenv.axon_hooks")
    holder = [None]
    mod.set_axon_ntff_profile_hook = lambda h: holder.__setitem__(0, h)
    mod.get_axon_ntff_profile_hook = lambda: holder[0]
    sys.modules["antenv.axon_hooks"] = mod
    antenv.axon_hooks = mod
    try:
        from trn_agent_boot.trn_boot import _ntff_profile_via_ctypes

        mod.set_axon_ntff_profile_hook(
            _ntff_profile_via_ctypes("/opt/axon/libaxon_pjrt.so"))
    except Exception as e:
        print(f"NTFF hook install failed: {e}")


def kernel(**inputs):
    global LAST_RESULTS
    from concourse.bass_utils import run_bass_kernel_spmd

    if "nc" not in _CACHE:
        _CACHE["nc"] = _build()
    nc = _CACHE["nc"]

    x = np.asarray(inputs["x"], dtype=np.float32)  # (8, 1024, 3)
    common = _prep_inputs({k: np.asarray(v) for k, v in inputs.items()})
    in_maps = []
    for i in range(NCORES):
        xT = np.ascontiguousarray(x[i].T)          # [3, 1024]
        x0i = np.zeros((33, N), np.float32)
        x0i[0:3] = xT
        x0i[32] = 1.0
        xa1 = np.zeros((33, N), np.float32)
        xa1[0:3] = xT
        xa1[32] = -0.5 * (xT * xT).sum(axis=0)
        in_maps.append(dict(common, x0in=x0i, xa1in=xa1))

    trace = bool(int(os.environ.get("DGCNN_TRACE", "0")))
    if trace:
        _install_ntff_hook()
    res = run_bass_kernel_spmd(nc, in_maps, core_ids=list(range(NCORES)),
                               trace=trace, trace_cores=[0] if trace else None)
    LAST_RESULTS = res
    out = np.stack([r["out"].reshape(40) for r in res.results]).astype(np.float32)
    return out


# revision 50
# speedup vs baseline: 1.0046x; 1.0046x over previous
"""DGCNN forward on 8 Trainium2 NeuronCores (Bass/Tile), pure data parallel.

V3: V2 restructured to cut DVE (vector engine) time, which the V2 trace
showed as the bottleneck (78% busy; MAX8+FIND_INDEX8+MATCH_REPLACE8 = 310us
of a 535us span):

- Packed-key top-k: the ACT engine quantizes distances to fp16
  (s = relu(-dist*sc + xx_i*sc + 2^-9), per-row bias via a PE transpose of
  the -0.5*xx row), one DVE STT builds key = (CONST - bits16(s))*1024 + j
  from the fp16 BIT PATTERN (monotone in -dist, log-spaced so resolution
  concentrates at near-zero distances where the top-20 boundary lives).
  3x max8 + 2x match_replace on keys then give values AND indices
  (idx = key mod 1024) -- the three FIND_INDEX8 passes and the u16->f32
  index CAST are gone. Keys are unique (j embedded) so match_replace is
  tie-safe.
- L4 neighbor-max tree: 6 wide contiguous fp16 TT ops (q-pair folding then
  a strided f-major j-tree) instead of 30 fragmented [128,256]/[128,128]
  ops.
- LeakyReLU moved to the ACT engine (Act.Lrelu); v tiles and the z=mk+v
  add are fp16 so the add runs in DVE 2x mode.
"""

import os

import numpy as np

N = 1024
K = 20
NCORES = 8
EPS = 1e-5
SLOPE = 0.01
NEG = -3.0e38

EDGE_LAYERS = [(3, 64), (64, 64), (64, 128), (128, 256)]
# Per-layer key scales: sc = 2/D, D = observed max |dist| with headroom.
SCALES = [2.0 / 64.4, 2.0 / 410.2, 2.0 / 161.4, 2.0 / 167.6]
KCONST = 15360.0  # key = (KCONST - bits16(s))*1024 + j; top keys < 2^24
SBIAS0 = 2.0 ** -9

_CACHE = {}
LAST_RESULTS = None
DIST_F32R = bool(int(os.environ.get("DGCNN_DIST_F32R", "1")))


def _build():
    import concourse.bass as bass
    import concourse.mybir as mybir
    import concourse.tile as tile
    from concourse import bacc

    dt = mybir.dt
    f32 = dt.float32
    u16 = dt.uint16
    f16 = dt.float16
    i16 = dt.int16
    i32 = dt.int32
    f32r = dt.float32r
    Alu = mybir.AluOpType
    Act = mybir.ActivationFunctionType
    AX = mybir.AxisListType

    nc = bacc.Bacc("TRN2", target_bir_lowering=False, debug=False,
                   num_swdge_queues=4)
    b32 = lambda ap: ap.bitcast(f32)

    # ---------------- DRAM I/O ----------------
    x0in = nc.dram_tensor("x0in", [33, N], f32, kind="ExternalInput")
    xa1in = nc.dram_tensor("xa1in", [33, N], f32, kind="ExternalInput")
    ATs, BTs, c0s = [], [], []
    for li, (C, O) in enumerate(EDGE_LAYERS):
        ATs.append(nc.dram_tensor(f"AT{li}", [C, O], f32, kind="ExternalInput"))
        BTs.append(nc.dram_tensor(f"BT{li}", [C, O], f32, kind="ExternalInput"))
        c0s.append(nc.dram_tensor(f"c0{li}", [1, O], f32, kind="ExternalInput"))
    w5T = nc.dram_tensor("w5T", [512, 1024], f32, kind="ExternalInput")
    l1T = nc.dram_tensor("l1T", [1024, 512], f32, kind="ExternalInput")
    b6 = nc.dram_tensor("b6", [1, 512], f32, kind="ExternalInput")
    l2T = nc.dram_tensor("l2T", [512, 256], f32, kind="ExternalInput")
    c7 = nc.dram_tensor("c7", [1, 256], f32, kind="ExternalInput")
    l3T = nc.dram_tensor("l3T", [256, 40], f32, kind="ExternalInput")
    b8 = nc.dram_tensor("b8", [1, 40], f32, kind="ExternalInput")
    identd = nc.dram_tensor("identd", [128, 128], f32, kind="ExternalInput")
    rep16d = nc.dram_tensor("rep16d", [16, 128], f32, kind="ExternalInput")
    initrd = nc.dram_tensor("initrd", [30, 1024], f32, kind="ExternalInput")
    iotad = nc.dram_tensor("iotad", [128, 1024], f32, kind="ExternalInput")
    out_d = nc.dram_tensor("out", [1, 40], f32, kind="ExternalOutput")

    with tile.TileContext(nc) as tc, __import__("contextlib").ExitStack() as ctx:
        const = ctx.enter_context(tc.tile_pool(name="const", bufs=1))
        xpool = ctx.enter_context(tc.tile_pool(name="xpool", bufs=1))
        work = ctx.enter_context(tc.tile_pool(name="work", bufs=2))
        gth_p = ctx.enter_context(tc.tile_pool(name="gth", bufs=10))
        vt_p = ctx.enter_context(tc.tile_pool(name="vt", bufs=3))
        small = ctx.enter_context(tc.tile_pool(name="small", bufs=4))
        pdp = ctx.enter_context(tc.tile_pool(name="pdp", bufs=2, space="PSUM"))
        mm = ctx.enter_context(tc.tile_pool(name="mm", bufs=2, space="PSUM"))
        fold_p = ctx.enter_context(tc.tile_pool(name="fold", bufs=1, space="PSUM"))
        dram = ctx.enter_context(tc.tile_pool(name="dram", bufs=2, space="DRAM"))

        # ------------- constants into SBUF (f32r via bitcast DMA) -------------
        def load_r2(name, dram_t, rows, cols):
            t = const.tile([rows, cols], f32r, tag=name)
            nc.scalar.dma_start(t[:], dram_t.ap().bitcast(f32r))
            return t

        AT_sb = [load_r2(f"AT{i}", ATs[i], *ATs[i].shape) for i in range(4)]
        BT_sb = [load_r2(f"BT{i}", BTs[i], *BTs[i].shape) for i in range(4)]
        c0_sb = [load_r2(f"c0{i}", c0s[i], *c0s[i].shape) for i in range(4)]
        b6_sb = load_r2("b6", b6, 1, 512)
        c7_sb = load_r2("c7", c7, 1, 256)
        b8_sb = load_r2("b8", b8, 1, 40)
        ident_s = const.tile([128, 128], f32, tag="ident_s")
        nc.sync.dma_start(ident_s[:], identd.ap())
        rep16_s = const.tile([16, 128], f32, tag="rep16_s")
        nc.sync.dma_start(rep16_s[:], rep16d.ap())
        iota_sb = const.tile([128, 1024], f32, tag="iota_sb")
        nc.sync.dma_start(iota_sb[:], iotad.ap())
        ones1024 = const.tile([1, 1024], f32r, tag="ones1024")
        nc.sync.dma_start(ones1024[:], initrd.ap().bitcast(f32r)[29:30, :])

        # point-major u tables in SBUF (SBUF-source SWDGE gather with
        # tokens_per_rank=128: idx j reads partition j%128, slot j//128)
        u0t = xpool.tile([128, 1024], f16, tag="u0")
        u1t = xpool.tile([128, 1024], f16, tag="u1")
        u2t = xpool.tile([128, 1024], f16, tag="u2")
        u3t = xpool.tile([128, 2048], f16, tag="u3t")
        u_sb = [u0t, u1t, u2t, u3t]

        def make_emit_u(li):
            C, O = EDGE_LAYERS[li]
            Opad = max(O, 128)

            def emit_u_pair(xch_next, qsl):
                uc = (lambda ap: ap) if O >= 256 else b32
                for mu in (qsl.start // 128, qsl.start // 128 + 1):
                    usl = slice(mu * 128, (mu + 1) * 128)
                    put = mm.tile([128, 512], f32, tag="mm")
                    pu = put[:, 0:O]
                    nc.tensor.matmul(pu, uc(xch_next[0:C, usl]),
                                     uc(AT_sb[li][:]), start=True, stop=False)
                    nc.tensor.matmul(pu, uc(ones_row[:]), uc(c0_sb[li][:]),
                                     start=False, stop=True)
                    nc.scalar.activation(
                        u_sb[li][:, mu * Opad:mu * Opad + O], pu, Act.Copy)

            return emit_u_pair

        # persistent channel-major feature tiles (f32r). lhsT tiles carry an
        # all-ones row after the feature rows (pairs with the rhs nxx row).
        x0 = xpool.tile([33, 1024], f32r, tag="x0")
        xar1 = xpool.tile([33, 1024], f32r, tag="xar1")
        x1 = xpool.tile([65, 1024], f32r, tag="x1")
        x2 = xpool.tile([65, 1024], f32r, tag="x2")
        x3 = xpool.tile([128, 1024], f32r, tag="x3")
        x4a = xpool.tile([128, 1024], f32r, tag="x4a")
        x4b = xpool.tile([128, 1024], f32r, tag="x4b")
        gp = xpool.tile([128, 8], f32, tag="gp")

        initr = initrd.ap().bitcast(f32r)
        nc.sync.dma_start(x0[:], x0in.ap().bitcast(f32r))
        nc.sync.dma_start(xar1[:], xa1in.ap().bitcast(f32r))
        # ones for rank-1 matmul tricks, f32r (initrd row 29 is all-ones)
        ones_row = const.tile([1, 128], f32r, tag="ones_row")
        nc.sync.dma_start(ones_row[:], initrd.ap().bitcast(f32r)[29:30, 0:128])
        ones_col = const.tile([128, 1], f32r, tag="ones_col")
        nc.sync.dma_start(ones_col[:],
                          initrd.ap().bitcast(f32r)[29:30, 0:128].rearrange("o c -> c o"))
        nc.sync.dma_start(x1[64:65, :], initr[29:30, :])  # ones row
        nc.sync.dma_start(x2[64:65, :], initr[29:30, :])  # ones row

        # ---------------- per-chunk prep for the NEXT layer ----------------
        def make_prep(xch_next, C, rhs_tile, emit_u_next=None):
            """Returns (per-chunk prep fn, nxxt-or-None). Emitted inside the
            previous layer's phase E so xsq/xar/colsum/nxx/u start as soon as
            each x chunk is written (engine streams are in-order)."""
            aug = C < 128
            xsq = work.tile([C, 1024], f32r, tag=f"xsq{C}", bufs=1)
            nxxt = None if aug else work.tile([1, 1024], f32r, tag="nxx", bufs=1)
            nxrow = (C if C >= 32 else 32) if aug else None

            def prep(mc):
                msl = slice(mc * 128, (mc + 1) * 128)
                nc.scalar.activation(xsq[0:C, msl], xch_next[0:C, msl],
                                     Act.Square)
                if aug:
                    nc.scalar.activation(rhs_tile[0:C, msl],
                                         xch_next[0:C, msl], Act.Copy)
                if mc % 2 == 1:
                    qsl = slice((mc - 1) * 128, (mc + 1) * 128)
                    pst = mm.tile([128, 512], f32, tag="mm")
                    ps = pst[0:1, 0:256]
                    nc.tensor.matmul(ps, b32(ones_col[0:C, :]),
                                     b32(xsq[0:C, qsl]))
                    dst = (rhs_tile[nxrow:nxrow + 1, qsl]
                           if aug else nxxt[0:1, qsl])
                    nc.scalar.activation(dst, ps, Act.Copy, scale=-0.5)
                    if emit_u_next is not None:
                        emit_u_next(xch_next, qsl)

            return prep, nxxt

        # ---------------- edge conv layer ----------------
        def edge_layer(li, xch, C, O, xouts, rhs_tile, nxxt=None, prep=None,
                       emit_u_self=None):
            """xch: lhsT tile ([C(+ones) rows, 1024], f32r).
            rhs_tile: None for L4 (uses xch + rank-1), else the rhs tile,
            filled by the previous layer's prep (L1: host-prebuilt xar1).
            prep: per-chunk prep fn for the NEXT layer, called in phase E.
            xouts: [(tile, rows)] per 128-channel output block."""
            Opad = max(O, 128)
            Of = Opad // 128
            aug = C < 128
            sc = SCALES[li]
            if aug:
                nxrow = 32 if li == 0 else 64
                nxap = rhs_tile[nxrow:nxrow + 1, :]
            else:
                nxrow = 0
                nxap = nxxt[0:1, :]
            id1 = ident_s[nxrow:nxrow + 1, nxrow:nxrow + 1]

            # v channel-major [O, 1024] (f16); deferred emission (phase E use)
            vs = []

            def emit_v():
                for f in range(Of if O >= 128 else 1):
                    osl = slice(f * 128, min((f + 1) * 128, O))
                    orows = osl.stop - osl.start
                    vt = vt_p.tile([128, 1024], f16, tag="vt")
                    for h in range(2):
                        nsl = slice(h * 512, (h + 1) * 512)
                        pv = mm.tile([128, 512], f32, tag="mm")
                        nc.tensor.matmul(pv[0:orows, :], BT_sb[li][:, osl],
                                         xch[0:C, nsl])
                        nc.scalar.activation(vt[0:orows, nsl], pv[0:orows, :],
                                             Act.Copy)
                    vs.append(vt)

            gq_tiles = {}

            def emit_gather(m):
                gq = gth_p.tile([128, Of * 2560], f16, tag=f"gth{Of}", bufs=3 if Of == 1 else 2)
                it, ioff = idx_tiles[m]
                for q in range(4):
                    dst = gq[:, q * Of * 640:(q + 1) * Of * 640]
                    nc.gpsimd.dma_gather(
                        dst.rearrange("p (f i) -> p f i", f=Of),
                        u_sb[li][:],
                        it[:, ioff + q * 40:ioff + (q + 1) * 40],
                        640, 640, Opad, transpose=True,
                        queue_num=q,
                        sbuf_tokens_per_rank=128,
                        sbuf_free_dim_per_rank=Opad * 2,
                    )
                gq_tiles[m] = gq

            # ---- phase E body: neighbor max (wide fp16 tree on DVE),
            #      z = mk + v (DVE, fp16 2x), lrelu on ACT; called one PAIR
            #      behind the topk loop so gq tiles recycle without a stall ----
            def phase_e(m):
                csl = slice(m * 128, (m + 1) * 128)
                gq = gq_tiles[m]

                def vmax(out, a, b):
                    nc.vector.tensor_tensor(out=out, in0=a, in1=b, op=Alu.max)

                if Of == 1:
                    # [128, 2560] = 20 contiguous j-slices of 128; wide tree
                    mkT = small.tile([128, 128], f16, tag="mkT")
                    a1 = small.tile([128, 1024], f16, tag="a1", bufs=2)
                    vmax(a1[:], gq[:, 0:1024], gq[:, 1024:2048])
                    vmax(a1[0:128, 0:512], a1[:, 0:512], a1[:, 512:1024])
                    vmax(a1[0:128, 0:256], a1[:, 0:256], a1[:, 256:512])
                    b1 = small.tile([128, 256], f16, tag="b1")
                    vmax(b1[:], gq[:, 2048:2304], gq[:, 2304:2560])
                    vmax(b1[0:128, 0:128], b1[:, 0:128], b1[:, 128:256])
                    vmax(a1[0:128, 0:128], a1[:, 0:128], a1[:, 128:256])
                    vmax(mkT[:], a1[:, 0:128], b1[:, 0:128])
                else:
                    # q-blocks [f2, j5, i128] of 1280: fold q-pairs with three
                    # wide ops, then a strided f-major j-tree (5 -> 1).
                    mkT = small.tile([128, 256], f16, tag="mk2")
                    a4 = small.tile([128, 1280], f16, tag="a4", bufs=1)
                    b4 = small.tile([128, 1280], f16, tag="b4", bufs=1)
                    vmax(a4[:], gq[:, 0:1280], gq[:, 1280:2560])
                    vmax(b4[:], gq[:, 2560:3840], gq[:, 3840:5120])
                    vmax(a4[:], a4[:], b4[:])
                    av = a4[:].rearrange("p (f j) -> p f j", f=2)
                    r1 = b4[:, 0:512].rearrange("p (f x) -> p f x", f=2)
                    vmax(r1, av[:, :, 0:256], av[:, :, 256:512])
                    r2 = b4[:, 512:768].rearrange("p (f x) -> p f x", f=2)
                    vmax(r2, r1[:, :, 0:128], r1[:, :, 128:256])
                    mkv = mkT[:].rearrange("p (f x) -> p f x", f=2)
                    vmax(mkv, r2, av[:, :, 512:640])

                for f, (xt, rows) in enumerate(xouts):
                    z = small.tile([128, 128], f16, tag="z")
                    nc.vector.tensor_add(z[0:rows, :],
                                         mkT[0:rows, f * 128:f * 128 + 128],
                                         vs[f][0:rows, csl])
                    nc.scalar.activation(xt[0:rows, csl], z[0:rows, :],
                                         Act.Lrelu, alpha=SLOPE)
                if prep is not None:
                    prep(m)

            # ---- phase B: dist + packed-key topk, two chunks interleaved ----
            idx_tiles = []
            dc = (lambda ap: ap) if DIST_F32R else (lambda ap: ap.bitcast(f32))

            def emit_dist_mm(m, btcol):
                csl = slice(m * 128, (m + 1) * 128)
                # per-row bias: sbias_i = xx_i*sc + 2^-9 from the -0.5*xx row
                nc.tensor.transpose(btcol, b32(nxap[0:1, csl]), id1)
                sbias = small.tile([128, 1], f32, tag="sb", bufs=4)
                nc.scalar.activation(sbias[:], btcol, Act.Copy,
                                     scale=-2.0 * sc, bias=SBIAS0)
                pd = pdp.tile([128, 1024], f32, tag="pd")
                s16 = work.tile([128, 1024], f16, tag="s16", bufs=3)
                for h in range(2):
                    nsl = slice(h * 512, (h + 1) * 512)
                    if aug:
                        nc.tensor.matmul(pd[:, nsl], dc(xch[:, csl]),
                                         dc(rhs_tile[:, nsl]))
                    else:
                        nc.tensor.matmul(pd[:, nsl], dc(xch[0:C, csl]),
                                         dc(xch[0:C, nsl]), start=True, stop=False)
                        nc.tensor.matmul(pd[:, nsl], dc(ones_row[:]),
                                         dc(nxxt[0:1, nsl]), start=False, stop=True)
                    nc.scalar.activation(s16[:, nsl], pd[:, nsl], Act.Relu,
                                         scale=-2.0 * sc, bias=sbias[:, 0:1])
                return s16

            def emit_keys(s16):
                keys = work.tile([128, 1024], f32, tag="keys", bufs=4)
                nc.vector.scalar_tensor_tensor(
                    out=keys[:], in0=s16[:].bitcast(u16), scalar=-1024.0,
                    in1=iota_sb[:], op0=Alu.mult, op1=Alu.add)
                return keys

            def emit_idx(m, idxf):
                # fold, permute-evac, replicate, convert (wrap order for SWDGE)
                psF = fold_p.tile([16, 160], f32, tag="psF")
                for t in range(8):
                    nc.tensor.matmul(psF[:, t * 20:(t + 1) * 20],
                                     ident_s[:, t * 16:(t + 1) * 16], idxf)
                wf = small.tile([16, 160], f32, tag="wf", bufs=2)
                nc.scalar.activation(
                    wf[:].rearrange("p (j h) -> p h j", h=8),
                    psF[:].rearrange("p (h j) -> p h j", h=8), Act.Copy)
                psRt = mm.tile([128, 512], f32, tag="mm")
                psR = psRt[:, 0:160]
                nc.tensor.matmul(psR, rep16_s[:], wf[:])
                idxw = work.tile([128, 160], i16, tag="idxw", bufs=9)
                nc.scalar.activation(idxw[:], psR, Act.Copy)
                idx_tiles.append((idxw, 0))
                emit_gather(m)

            pend = None
            for mp in range(4):
                m0, m1 = 2 * mp, 2 * mp + 1
                if mp == 0:
                    bt = fold_p.tile([128, 4], f32, tag="ph")
                    pend = (emit_dist_mm(m0, bt[:, 0:1]),
                            emit_dist_mm(m1, bt[:, 1:2]))
                keys0 = emit_keys(pend[0])
                keys1 = emit_keys(pend[1])
                m48 = small.tile([128, 48], f32, tag="m48", bufs=4)
                for r in range(3):
                    rsl0 = slice(r * 8, (r + 1) * 8)
                    rsl1 = slice(24 + r * 8, 24 + (r + 1) * 8)
                    nc.vector.max(m48[:, rsl0], keys0[:])
                    nc.vector.max(m48[:, rsl1], keys1[:])
                    if r < 2:
                        nc.vector.match_replace(keys0[:], m48[:, rsl0],
                                                keys0[:], NEG)
                        nc.vector.match_replace(keys1[:], m48[:, rsl1],
                                                keys1[:], NEG)
                # hoist the next pair's dist matmuls + fp16 quantize (PE/ACT
                # only) ahead of this pair's idx/tree ACT work so s16 is
                # ready when the next keys-STT issues on the DVE
                if mp < 3:
                    btn = fold_p.tile([128, 4], f32, tag="ph")
                    pend = (emit_dist_mm(m0 + 2, btn[:, 0:1]),
                            emit_dist_mm(m1 + 2, btn[:, 1:2]))
                # idx = key & 1023 via exact i32 cast (keys are exact ints);
                # both chunks' 20 indices extracted in one 3-op sequence
                idxf2 = small.tile([128, 40], f32, tag="idxf", bufs=4)
                ti = small.tile([128, 40], i32, tag="ti", bufs=4)
                nc.vector.tensor_copy(
                    ti[:].rearrange("p (c j) -> p c j", c=2),
                    m48[:].rearrange("p (c x) -> p c x", c=2)[:, :, 0:20])
                nc.vector.tensor_scalar(ti[:], ti[:], 1023, None,
                                        op0=Alu.bitwise_and)
                nc.vector.tensor_copy(idxf2[:], ti[:])
                idxf0 = idxf2[:, 0:20]
                idxf1 = idxf2[:, 20:40]
                if mp == 0 and emit_u_self is not None:
                    for up in range(4):
                        emit_u_self(xch, slice(up * 256, (up + 1) * 256))
                emit_idx(m0, idxf0)
                emit_idx(m1, idxf1)
                if mp == 0:
                    emit_v()
                if mp >= 1:
                    phase_e(2 * (mp - 1))
                    phase_e(2 * mp - 1)
            phase_e(6)
            phase_e(7)

        xar2 = xpool.tile([65, 1024], f32r, tag="xar2")
        xar3 = xpool.tile([65, 1024], f32r, tag="xar3")
        prep2, _ = make_prep(x1, 64, xar2, emit_u_next=make_emit_u(1))
        edge_layer(0, x0, 3, 64, [(x1, 64)], xar1, prep=prep2,
                   emit_u_self=make_emit_u(0))
        prep3, _ = make_prep(x2, 64, xar3, emit_u_next=make_emit_u(2))
        edge_layer(1, x1, 64, 64, [(x2, 64)], xar2, prep=prep3)
        prep4, nxxt4 = make_prep(x3, 128, None, emit_u_next=make_emit_u(3))
        edge_layer(2, x2, 64, 128, [(x3, 128)], xar3, prep=prep4)

        # conv5 weights + head weights (loaded during L3/L4)
        w5_rows = [(0, 64), (64, 128), (128, 256), (256, 384), (384, 512)]
        w5_sb = []
        for i, (r0, r1) in enumerate(w5_rows):
            t = const.tile([r1 - r0, 1024], f32r, tag=f"w5_{i}")
            nc.sync.dma_start(t[:], w5T.ap().bitcast(f32r)[r0:r1, :])
            w5_sb.append(t)

        # conv5 emitted inside L4's phase E: half 0 after chunk 3's features,
        # half 1 after chunk 7 — PE chews it while DVE finishes L4 topk/trees
        xc_full = [(x1, 64), (x2, 64), (x3, 128), (x4a, 128), (x4b, 128)]
        gph = xpool.tile([128, 32], f32, tag="gph")

        def conv5_q(q):
            nsl = slice(q * 256, (q + 1) * 256)
            for mo in range(8):
                msl = slice(mo * 128, (mo + 1) * 128)
                pet = mm.tile([128, 512], f32, tag="mm")
                pe = pet[:, 0:256]
                for k in range(5):
                    nc.tensor.matmul(pe, w5_sb[k][:, msl],
                                     xc_full[k][0][0:xc_full[k][1], nsl],
                                     start=(k == 0), stop=(k == 4))
                nc.vector.reduce_max(gph[:, q * 8 + mo:q * 8 + mo + 1], pe,
                                     axis=AX.X)

        def prep5(mc):
            if mc % 2 == 1:
                conv5_q(mc // 2)

        edge_layer(3, x3, 128, 256, [(x4a, 128), (x4b, 128)], None,
                   nxxt=nxxt4, prep=prep5)
        gpt = small.tile([128, 16], f32, tag="gpt", bufs=1)
        nc.vector.tensor_tensor(out=gpt[:, 0:8], in0=gph[:, 0:8],
                                in1=gph[:, 8:16], op=Alu.max)
        nc.vector.tensor_tensor(out=gpt[:, 8:16], in0=gph[:, 16:24],
                                in1=gph[:, 24:32], op=Alu.max)
        nc.vector.tensor_tensor(out=gp[:, 0:8], in0=gpt[:, 0:8],
                                in1=gpt[:, 8:16], op=Alu.max)

        l1_sb = []
        for k in range(8):
            t = const.tile([128, 512], f32r, tag=f"l1_{k}")
            nc.sync.dma_start(t[:], l1T.ap().bitcast(f32r)[k * 128:(k + 1) * 128, :])
            l1_sb.append(t)
        l2_sb = []
        for k in range(4):
            t = const.tile([128, 256], f32r, tag=f"l2_{k}")
            nc.sync.dma_start(t[:], l2T.ap().bitcast(f32r)[k * 128:(k + 1) * 128, :])
            l2_sb.append(t)
        l3_sb = []
        for k in range(2):
            t = const.tile([128, 40], f32r, tag=f"l3_{k}")
            nc.sync.dma_start(t[:], l3T.ap().bitcast(f32r)[k * 128:(k + 1) * 128, :])
            l3_sb.append(t)

        # ---------------- MLP head: row-form GEMV (f32r, N=512/256) ----------
        gpr = small.tile([128, 8], f32r, tag="gpr", bufs=1)
        nc.scalar.activation(gpr[:], gp[:], Act.Copy)

        p1t = mm.tile([128, 512], f32, tag="mm")
        p1r = p1t[0:1, :]
        for k in range(8):
            nc.tensor.matmul(p1r, gpr[:, k:k + 1], l1_sb[k][:],
                             start=(k == 0), stop=False)
        nc.tensor.matmul(p1r, ones_row[0:1, 0:1], b6_sb[:],
                         start=False, stop=True)
        y1l = small.tile([1, 512], f32, tag="y1l", bufs=1)
        nc.scalar.activation(y1l[:], p1r, Act.Lrelu, alpha=SLOPE)
        p1c = fold_p.tile([128, 4], f32, tag="ph")
        for j in range(4):
            nc.tensor.transpose(p1c[:, j:j + 1], y1l[0:1, j * 128:(j + 1) * 128],
                                ident_s[0:1, 0:1])
        y1c = small.tile([128, 4], f32r, tag="y1c", bufs=1)
        nc.scalar.activation(y1c[:], p1c[:], Act.Copy)

        p2t = mm.tile([128, 512], f32, tag="mm")
        p2r = p2t[0:1, 0:256]
        for k in range(4):
            nc.tensor.matmul(p2r, y1c[:, k:k + 1], l2_sb[k][:],
                             start=(k == 0), stop=False)
        nc.tensor.matmul(p2r, ones_row[0:1, 0:1], c7_sb[:],
                         start=False, stop=True)
        y2l = small.tile([1, 256], f32, tag="y2l", bufs=1)
        nc.scalar.activation(y2l[:], p2r, Act.Lrelu, alpha=SLOPE)
        p2c = fold_p.tile([128, 4], f32, tag="ph")
        for j in range(2):
            nc.tensor.transpose(p2c[:, j:j + 1], y2l[0:1, j * 128:(j + 1) * 128],
                                ident_s[0:1, 0:1])
        y2c = small.tile([128, 2], f32, tag="y2c", bufs=1)
        nc.scalar.activation(y2c[:], p2c[:, 0:2], Act.Copy)

        p3t = mm.tile([128, 512], f32, tag="mm")
        p3r = p3t[0:1, 0:40]
        for k in range(2):
            nc.tensor.matmul(p3r, y2c[:, k:k + 1], b32(l3_sb[k][:, 0:40]),
                             start=(k == 0), stop=False)
        nc.tensor.matmul(p3r, b32(ones_row[0:1, 0:1]), b32(b8_sb[:]),
                         start=False, stop=True)
        y3 = small.tile([1, 40], f32, tag="y3", bufs=1)
        nc.scalar.activation(y3[:], p3r, Act.Copy)
        nc.sync.dma_start(out_d.ap(), y3[:])

    nc.compile()
    return nc


def _prep_inputs(inputs):
    """Fold eval-mode BN into conv/linear weights; transpose for the device."""
    f = np.float32
    s = lambda g: (g / np.sqrt(f(1.0) + f(EPS))).astype(f)

    def edge(w, g, b, bias=None):
        O, C2 = w.shape
        C = C2 // 2
        sc = s(g)
        Wd = w[:, :C]
        Wc = w[:, C:]
        A = sc[:, None] * Wd
        Bm = sc[:, None] * (Wc - Wd)
        c0 = sc * (bias if bias is not None else 0.0) + b
        return A.T.copy().astype(f), Bm.T.copy().astype(f), c0.reshape(1, -1).astype(f)

    d = {}
    d["AT0"], d["BT0"], d["c00"] = edge(inputs["conv1_w"], inputs["bn1_g"],
                                        inputs["bn1_b"], inputs["conv1_b"])
    d["AT1"], d["BT1"], d["c01"] = edge(inputs["conv2_w"], inputs["bn2_g"], inputs["bn2_b"])
    d["AT2"], d["BT2"], d["c02"] = edge(inputs["conv3_w"], inputs["bn3_g"], inputs["bn3_b"])
    d["AT3"], d["BT3"], d["c03"] = edge(inputs["conv4_w"], inputs["bn4_g"], inputs["bn4_b"])
    d["w5T"] = inputs["conv5_w"].T.copy().astype(f)
    s6 = s(inputs["bn6_g"])
    d["l1T"] = (s6[:, None] * inputs["lin1_w"]).T.copy().astype(f)
    d["b6"] = inputs["bn6_b"].reshape(1, -1).astype(f)
    s7 = s(inputs["bn7_g"])
    d["l2T"] = (s7[:, None] * inputs["lin2_w"]).T.copy().astype(f)
    d["c7"] = (s7 * inputs["lin2_b"] + inputs["bn7_b"]).reshape(1, -1).astype(f)
    d["l3T"] = inputs["lin3_w"].T.copy().astype(f)
    d["b8"] = inputs["lin3_b"].reshape(1, -1).astype(f)
    d["identd"] = np.eye(128, dtype=f)
    rep = np.zeros((16, 128), f)
    for m in range(128):
        rep[m % 16, m] = 1.0
    d["rep16d"] = rep
    initr = np.zeros((30, 1024), f)
    initr[29, :] = 1.0
    d["initrd"] = initr
    iota = (KCONST * 1024.0 + np.arange(1024)).astype(f)
    d["iotad"] = np.broadcast_to(iota, (128, 1024)).copy()
    return d


def _install_ntff_hook():
    import sys
    import types

    if "antenv.axon_hooks" in sys.modules:
        return
    import antenv

    mod = types.ModuleType("antenv.axon_hooks")
    holder = [None]
    mod.set_axon_ntff_profile_hook = lambda h: holder.__setitem__(0, h)
    mod.get_axon_ntff_profile_hook = lambda: holder[0]
    sys.modules["antenv.axon_hooks"] = mod
    antenv.axon_hooks = mod
    try:
        from trn_agent_boot.trn_boot import _ntff_profile_via_ctypes

        mod.set_axon_ntff_profile_hook(
            _ntff_profile_via_ctypes("/opt/axon/libaxon_pjrt.so"))
    except Exception as e:
        print(f"NTFF hook install failed: {e}")


def kernel(**inputs):
    global LAST_RESULTS
    from concourse.bass_utils import run_bass_kernel_spmd

    if "nc" not in _CACHE:
        _CACHE["nc"] = _build()
    nc = _CACHE["nc"]

    x = np.asarray(inputs["x"], dtype=np.float32)  # (8, 1024, 3)
    common = _prep_inputs({k: np.asarray(v) for k, v in inputs.items()})
    in_maps = []
    for i in range(NCORES):
        xT = np.ascontiguousarray(x[i].T)          # [3, 1024]
        x0i = np.zeros((33, N), np.float32)
        x0i[0:3] = xT
        x0i[32] = 1.0
        xa1 = np.zeros((33, N), np.float32)
        xa1[0:3] = xT
        xa1[32] = -0.5 * (xT * xT).sum(axis=0)
        in_maps.append(dict(common, x0in=x0i, xa1in=xa1))

    trace = bool(int(os.environ.get("DGCNN_TRACE", "0")))
    if trace:
        _install_ntff_hook()
    res = run_bass_kernel_spmd(nc, in_maps, core_ids=list(range(NCORES)),
                               trace=trace, trace_cores=[0] if trace else None)
    LAST_RESULTS = res
    out = np.stack([r["out"].reshape(40) for r in res.results]).astype(np.float32)
    return out
